# revision 5
# baseline (speedup 1.0000x reference)
"""CornerPool kernel for Trainium2 — fused 1D Winograd F(2,3) along H, bf16.

One sample per NeuronCore (B=8). All 3x3 convs use Winograd F(2,3) on the
H axis (2 output rows per tile, taps along W stay direct): per output
chunk of 8 rows, 4 PSUM banks accumulate M_u = sum_{ci,dx} U_u^T V_u with
U_u = G-transformed (BN-folded) weights; DVE combines y_even=M0+M1+M2,
y_odd=M1-M2-M3; ScalarE applies bias+ReLU. The 1x1 convs of stage C are
folded into the M0 (+w1) and M3 (-w1) accumulations, so they ride the
same inverse. Corner pools: H pools via in-place shifted-max doubling
on DVE + inter-chunk carry; W pools via DVE prefix-scan per row. A short
burst of dummy matmuls on zeroed scratch warms the PE clock during the
preamble DMA wait. The whole net runs fused in
SBUF (two directional passes: TL descending, BR ascending) — x, weights
and rolling sum/tl windows stay on-chip; only x/weights in and outputs
out touch DRAM.
"""

import numpy as np

_P = 128
_CH = 16          # chunks per image; chunk = 8 image rows = 4 Winograd tiles
_G = np.array([[1, 0, 0], [0.5, 0.5, 0.5], [0.5, -0.5, 0.5], [0, 0, 1]],
              np.float32)


def _bf16():
    import ml_dtypes
    return ml_dtypes.bfloat16


def _prep_host(inputs):
    """Fold BN scales, G-transform weights along dy, build bf16 lhsT arrays."""
    f32 = np.float32
    BF = _bf16()

    def scaled(name):
        w = np.asarray(inputs["w_" + name], f32)
        s = np.asarray(inputs["s_" + name], f32)
        return w * s[:, None, None, None]

    def bias(name):
        return np.asarray(inputs["b_" + name], f32)

    def gtrans(w):
        # w [co, ci, 3, 3] -> [ci, 4u, 3dx, co]
        return np.einsum('uy,oiyx->iuxo', _G, w).astype(f32)

    # stage A: [ci=256, 4, 3, co=128] -> [4conv][128k, 2ci*12, 128m]
    def layA(w):
        a = gtrans(w).reshape(2, 128, 12, 128)
        return np.ascontiguousarray(a.transpose(1, 0, 2, 3).reshape(128, 24, 128))

    ua = np.stack([layA(scaled(n)) for n in ("t", "l", "b", "r")]).astype(BF)

    # stage C3: [ci=128, 4, 3, co=256] -> [2br][128k, 2co*12, 128m]
    def layC(w3):
        a = gtrans(w3).reshape(128, 12, 2, 128)
        return np.ascontiguousarray(a.transpose(0, 2, 1, 3).reshape(128, 24, 128))

    uc = np.stack([layC(scaled("tl3")), layC(scaled("br3"))]).astype(BF)

    # stage C1: [co=256, ci=256] -> [2br][128k, co_t*4 + sign*2 + ci_t, 128m]
    def layC1(w1):
        a = w1[:, :, 0, 0].T.reshape(2, 128, 2, 128)   # ci_t, k, co_t, m
        both = np.stack([a, -a], axis=0)               # sign, ci_t, k, co_t, m
        return np.ascontiguousarray(
            both.transpose(2, 3, 0, 1, 4).reshape(128, 8, 128))

    w1 = np.stack([layC1(scaled("tl1")), layC1(scaled("br1"))]).astype(BF)

    # stage D: [ci=256, 4, 3, co=256] -> [2br][128k, co_t*24 + ci_t*12 + uxdx, 128m]
    def layD(w):
        a = gtrans(w).reshape(2, 128, 12, 2, 128)      # ci_t, k, uxdx, co_t, m
        return np.ascontiguousarray(
            a.transpose(1, 3, 0, 2, 4).reshape(128, 48, 128))

    ud = np.stack([layD(scaled("tlo")), layD(scaled("bro"))]).astype(BF)

    bias_rows = [bias("t"), bias("l"), bias("b"), bias("r")]
    for n3, n1 in (("tl3", "tl1"), ("br3", "br1")):
        comb = bias(n3) + bias(n1)
        bias_rows += [comb[:128], comb[128:]]
    for n in ("tlo", "bro"):
        bb = bias(n)
        bias_rows += [bb[:128], bb[128:]]
    bias_all = np.ascontiguousarray(np.stack(bias_rows).T).astype(f32)

    return {"ua": ua, "uc": uc, "w1": w1, "ud": ud, "bias": bias_all}


def _pad_x_sample(xs, H):
    """[256, H, 128] f32 -> [2, 128, H+2, 130] bf16 zero-padded."""
    BF = _bf16()
    xp = np.zeros((2, 128, H + 2, 130), BF)
    xp[:, :, 1:H + 1, 1:129] = xs.reshape(2, 128, H, 128).astype(BF)
    return xp


def _build(H):
    import concourse.bacc as bacc
    import concourse.mybir as mybir
    import concourse.tile as tile
    import contextlib

    dt = mybir.dt
    Alu = mybir.AluOpType
    Act = mybir.ActivationFunctionType
    BF = dt.bfloat16
    CH = H // 8
    HP = H + 2

    nc = bacc.Bacc("TRN2", target_bir_lowering=False, debug=False)

    xpad = nc.dram_tensor("xpad", [2, 128, HP, 130], BF, kind="ExternalInput")
    ua_d = nc.dram_tensor("ua", [4, 128, 24, 128], BF, kind="ExternalInput")
    uc_d = nc.dram_tensor("uc", [2, 128, 24, 128], BF, kind="ExternalInput")
    w1_d = nc.dram_tensor("w1", [2, 128, 8, 128], BF, kind="ExternalInput")
    ud_d = nc.dram_tensor("ud", [2, 128, 48, 128], BF, kind="ExternalInput")
    bias_d = nc.dram_tensor("bias", [128, 12], dt.float32, kind="ExternalInput")
    # outputs declared row-parity-split: [co, jj, t, w] = [co, 2*jj + t, w]
    out_tl = nc.dram_tensor("out_tl", [256, H // 2, 2, 128], dt.float32,
                            kind="ExternalOutput")
    out_br = nc.dram_tensor("out_br", [256, H // 2, 2, 128], dt.float32,
                            kind="ExternalOutput")
    outs = [out_tl, out_br]

    with tile.TileContext(nc) as tc:
        with contextlib.ExitStack() as ctx:
            xpool = ctx.enter_context(tc.tile_pool(name="xp", bufs=1))
            upool = ctx.enter_context(tc.tile_pool(name="up", bufs=1))
            rpool = ctx.enter_context(tc.tile_pool(name="rp", bufs=1))
            vpool = ctx.enter_context(tc.tile_pool(name="vp", bufs=1))
            tpool = ctx.enter_context(tc.tile_pool(name="tp", bufs=2))
            ipool = ctx.enter_context(tc.tile_pool(name="ip", bufs=3))
            opool = ctx.enter_context(tc.tile_pool(name="op", bufs=2))
            cpool = ctx.enter_context(tc.tile_pool(name="cp", bufs=2))
            mpool = ctx.enter_context(tc.tile_pool(name="mp", bufs=1))
            pspool = ctx.enter_context(tc.tile_pool(name="ps", bufs=2,
                                                    space="PSUM"))

            # ---------------- preamble: x, weights, rings ----------------
            xt = [xpool.tile([128, HP, 130], BF, tag=f"x{c}", name=f"x{c}")
                  for c in (0, 1)]
            bt = mpool.tile([128, 12], dt.float32, tag="bias")
            nc.gpsimd.dma_start(bt[:], bias_d.ap())

            # PE warm-up: dummy matmuls on zeroed scratch fill the preamble
            # DMA wait so the HAM clock ramp is paid on idle time and the
            # first real matmuls run at full rate.
            nwarm = int(__import__("os").environ.get("NWARM", "30"))
            if nwarm:
                wsc = mpool.tile([128, 512], BF, tag="warm", name="wsc")
                nc.vector.memset(wsc[:].bitcast(dt.uint16), 0.0)
                wps = pspool.tile([128, 512], dt.float32, tag="m0",
                                  name="m0w")
                for _ in range(nwarm):
                    nc.tensor.matmul(wps[:], wsc[:, 0:128], wsc[:],
                                     start=True, stop=True)

            def load_wb(bi):
                """Load one branch's transformed weights into SBUF."""
                a_t, a_l = (0, 1) if bi == 0 else (2, 3)
                uat = upool.tile([128, 24, 128], BF, tag="uat", name="uat",
                                 bufs=2)
                ual = upool.tile([128, 24, 128], BF, tag="ual", name="ual",
                                 bufs=2)
                ucb = upool.tile([128, 24, 128], BF, tag="ucb", name="ucb")
                w1b = upool.tile([128, 8, 128], BF, tag="w1b", name="w1b")
                udb = upool.tile([128, 48, 128], BF, tag="udb", name="udb")
                nc.gpsimd.dma_start(uat[:], ua_d.ap()[a_t])
                nc.gpsimd.dma_start(ual[:], ua_d.ap()[a_l])
                nc.gpsimd.dma_start(ucb[:], uc_d.ap()[bi])
                nc.gpsimd.dma_start(w1b[:], w1_d.ap()[bi])
                nc.gpsimd.dma_start(udb[:], ud_d.ap()[bi])
                return {a_t: uat, a_l: ual}, ucb, w1b, udb

            # x lands in need-order for the first (descending) branch:
            # top 10 rows (first chunk) -> stage-A weights -> the rest in
            # progressively larger slices. The DMA pool drains serially, so
            # byte order here is the PE-start latency.
            for c in (0, 1):
                eng = nc.sync if c == 0 else nc.scalar
                eng.dma_start(xt[c][:, HP - 10:, :], xpad.ap()[c][:, HP - 10:, :])
            wb0 = load_wb(0)
            cuts = [HP - 10, HP - 42, HP - 74, 0]
            for i in range(len(cuts) - 1):
                a, b = cuts[i + 1], cuts[i]
                for c in (0, 1):
                    eng = nc.sync if c == 0 else nc.scalar
                    eng.dma_start(xt[c][:, a:b, :], xpad.ap()[c][:, a:b, :])

            # persistent ring tiles (10 rows = 8 + 2 halo), zeroed once
            st = [rpool.tile([128, 10, 130], BF, tag=f"s{r}", name=f"st{r}")
                  for r in range(3)]
            tlt = [[rpool.tile([128, 10, 130], BF, tag=f"t{co}{r}",
                               name=f"tlt{co}{r}")
                    for r in range(3)] for co in range(2)]
            for tile_ in st:
                nc.vector.memset(tile_[:].bitcast(dt.uint16), 0.0)
            for tile_ in tlt[0] + tlt[1]:
                nc.gpsimd.memset(tile_[:].bitcast(dt.uint16), 0.0)

            # parity views: rows (5,2): [p, j, par, w]
            def par(t10):
                return t10[:].rearrange("p (j t) w -> p j t w", t=2)

            xr = [t[:].rearrange("p (j t) w -> p j t w", t=2) for t in xt]

            def inv_group(mg, brow, dst_e, dst_o):
                """ACT evacuates the 4-bank M group in one copy to bf16 SBUF;
                DVE combines (2x); ACT applies bias+ReLU to both parities."""
                s = ipool.tile([128, 4, 512], BF, tag="ev", name="ev")
                for u in range(4):
                    nc.scalar.copy(s[:, u], mg[u][:])
                a = ipool.tile([128, 512], BF, tag="cmb0", name="cmb0")
                b = ipool.tile([128, 512], BF, tag="cmb1", name="cmb1")
                nc.vector.tensor_tensor(a[:], s[:, 0], s[:, 1], Alu.add)
                nc.vector.tensor_tensor(a[:], a[:], s[:, 2], Alu.add)
                nc.vector.tensor_tensor(b[:], s[:, 1], s[:, 2], Alu.subtract)
                nc.vector.tensor_tensor(b[:], b[:], s[:, 3], Alu.subtract)
                r = lambda t: t[:].rearrange("p (a b) -> p a b", b=128)
                nc.scalar.activation(dst_e, r(a), Act.Relu,
                                     bias=bt[:, brow:brow + 1], scale=1.0)
                nc.scalar.activation(dst_o, r(b), Act.Relu,
                                     bias=bt[:, brow:brow + 1], scale=1.0)

            def vmaps(src_par, j0, pfx, n=4):
                """4 V tiles [128, n, 130] from parity view at tile row j0."""
                vt = [vpool.tile([128, 4, 130], BF, tag=f"v{pfx}{u}",
                                 name=f"v{pfx}{u}")
                      for u in range(4)]
                e0 = src_par[:, j0:j0 + n, 0, :]
                o0 = src_par[:, j0:j0 + n, 1, :]
                e1 = src_par[:, j0 + 1:j0 + n + 1, 0, :]
                o1 = src_par[:, j0 + 1:j0 + n + 1, 1, :]
                nc.vector.tensor_tensor(vt[0][:, :n], e0, e1, Alu.subtract)
                nc.vector.tensor_tensor(vt[1][:, :n], o0, e1, Alu.add)
                nc.vector.tensor_tensor(vt[2][:, :n], e1, o0, Alu.subtract)
                nc.vector.tensor_tensor(vt[3][:, :n], o0, o1, Alu.subtract)
                return vt

            def mgroup():
                g = [pspool.tile([128, 512], dt.float32, tag=f"m{u}",
                                 name=f"m{u}")
                     for u in range(4)]
                return g, g

            def wino_mms(mm, vt, w, off, start, stop_banks):
                """Accumulate 4 banks: mm[u] += sum_dx w[:, off+u*3+dx]^T vt[u]."""
                for u in range(4):
                    for dx in range(3):
                        nc.tensor.matmul(
                            mm[u][:], w[:, off + u * 3 + dx],
                            vt[u][:, :, dx:dx + 128],
                            start=start and dx == 0,
                            stop=(u in stop_banks) and dx == 2)

            # ------------------------- branch pass -----------------------
            def branch(bi, desc):
                a_t, a_l = (0, 1) if bi == 0 else (2, 3)
                order = list(reversed(range(CH))) if desc else list(range(CH))
                ua2, ucb, w1b, udb = wb0 if bi == 0 else load_wb(1)
                carry = cpool.tile([128, 1, 128], BF, tag=f"cr{bi}")
                nc.vector.memset(carry[:].bitcast(dt.uint16), 0.0)

                def stageA(k, p):
                    nonlocal carry
                    # V of x for this chunk: padded rows 8k..8k+9 -> j0 = 4k
                    vx = [vmaps(xr[c], 4 * k, f"x{c}") for c in (0, 1)]
                    tl_tiles = []
                    for conv in (a_t, a_l):
                        mm, mg = mgroup()
                        for c in (0, 1):
                            wino_mms(mm, vx[c], ua2[conv], 12 * c,
                                     start=c == 0,
                                     stop_banks=(0, 1, 2, 3) if c == 1 else ())
                        tt = tpool.tile([128, 8, 130], BF, tag=f"ab{conv % 2}")
                        tpar = tt[:].rearrange("p (j t) w -> p j t w", t=2)
                        inv_group(mg, conv, tpar[:, 0:4, 0, 1:129],
                                  tpar[:, 0:4, 1, 1:129])
                        tl_tiles.append(tt)
                    tt, lt = tl_tiles
                    # H pool on tt: in-place shifted maxes (DVE streaming)
                    ti = tt[:, :, 1:129]
                    if desc:
                        nc.vector.tensor_tensor(ti[:, 0:7], ti[:, 0:7],
                                                ti[:, 1:8], Alu.max)
                        nc.vector.tensor_tensor(ti[:, 0:6], ti[:, 0:6],
                                                ti[:, 2:8], Alu.max)
                        nc.vector.tensor_tensor(ti[:, 0:4], ti[:, 0:4],
                                                ti[:, 4:8], Alu.max)
                    else:
                        nc.vector.tensor_tensor(ti[:, 1:8], ti[:, 1:8],
                                                ti[:, 0:7], Alu.max)
                        nc.vector.tensor_tensor(ti[:, 2:8], ti[:, 2:8],
                                                ti[:, 0:6], Alu.max)
                        nc.vector.tensor_tensor(ti[:, 4:8], ti[:, 4:8],
                                                ti[:, 0:4], Alu.max)
                    nc.vector.tensor_tensor(ti[:], ti[:],
                                            carry[:].broadcast_to([128, 8, 128]),
                                            Alu.max)
                    if p != CH - 1:
                        nxt = cpool.tile([128, 1, 128], BF, tag=f"cr{bi}")
                        csrc = ti[:, 0:1] if desc else ti[:, 7:8]
                        nc.vector.tensor_copy(nxt[:], csrc)
                        carry = nxt
                    # W pool on lt (reverse for TL, forward for BR)
                    for h in range(8):
                        v = lt[:, h, 1:129]
                        if bi == 0:
                            v = v[:, ::-1]
                        nc.vector.tensor_tensor_scan(v, v, v, 0.0,
                                                     op0=Alu.max, op1=Alu.bypass)
                    # sum -> ring tile k (rows 1..8 interior)
                    s = st[k % 3]
                    nc.vector.tensor_tensor(s[:, 1:9, 1:129], ti[:],
                                            lt[:, :, 1:129], Alu.add)
                    # halo exchange with previously-produced neighbor
                    nb = k + 1 if desc else k - 1
                    if p == 0:
                        edge = s[:, 9:10, :] if desc else s[:, 0:1, :]
                        nc.gpsimd.memset(edge.bitcast(dt.uint16), 0.0)
                    else:
                        nbt = st[nb % 3]
                        if desc:  # my row8 -> nb row0 ; nb row1 -> my row9
                            nc.vector.tensor_copy(nbt[:, 0:1, :], s[:, 8:9, :])
                            nc.vector.tensor_copy(s[:, 9:10, :], nbt[:, 1:2, :])
                        else:     # my row1 -> nb row9 ; nb row8 -> my row0
                            nc.vector.tensor_copy(nbt[:, 9:10, :], s[:, 1:2, :])
                            nc.vector.tensor_copy(s[:, 0:1, :], nbt[:, 8:9, :])
                    if p == CH - 1:
                        edge = s[:, 0:1, :] if desc else s[:, 9:10, :]
                        nc.gpsimd.memset(edge.bitcast(dt.uint16), 0.0)

                def stageC(j):
                    s = st[j % 3]
                    vs = vmaps(par(s), 0, "s")
                    for co in range(2):
                        mm, mg = mgroup()
                        wino_mms(mm, vs, ucb, 12 * co,
                                 start=True, stop_banks=(1, 2))
                        # fold C1: +w1 into m0 (even), -w1 into m3 (odd)
                        for c in (0, 1):
                            nc.tensor.matmul(
                                mm[0][:], w1b[:, co * 4 + 0 * 2 + c],
                                xr[c][:, 4 * j:4 * j + 4, 1, 1:129],
                                start=False, stop=c == 1)
                            nc.tensor.matmul(
                                mm[3][:], w1b[:, co * 4 + 1 * 2 + c],
                                xr[c][:, 4 * j + 1:4 * j + 5, 0, 1:129],
                                start=False, stop=c == 1)
                        d = tlt[co][j % 3]
                        dpar = par(d)
                        brow = 4 + bi * 2 + co
                        inv_group(mg, brow, dpar[:, 0:4, 1, 1:129],
                                  dpar[:, 1:5, 0, 1:129])
                    # halo exchange on tl ring
                    nb = j + 1 if desc else j - 1
                    first = (j == order[0])
                    last = (j == order[-1])
                    for co in range(2):
                        d = tlt[co][j % 3]
                        if first:
                            edge = d[:, 9:10, :] if desc else d[:, 0:1, :]
                            nc.gpsimd.memset(edge.bitcast(dt.uint16), 0.0)
                        else:
                            nbt = tlt[co][nb % 3]
                            if desc:
                                nc.vector.tensor_copy(nbt[:, 0:1, :], d[:, 8:9, :])
                                nc.vector.tensor_copy(d[:, 9:10, :], nbt[:, 1:2, :])
                            else:
                                nc.vector.tensor_copy(nbt[:, 9:10, :], d[:, 1:2, :])
                                nc.vector.tensor_copy(d[:, 0:1, :], nbt[:, 8:9, :])
                        if last:
                            edge = d[:, 0:1, :] if desc else d[:, 9:10, :]
                            nc.gpsimd.memset(edge.bitcast(dt.uint16), 0.0)

                def stageD(j):
                    vt = [vmaps(par(tlt[c][j % 3]), 0, f"d{c}") for c in (0, 1)]
                    orr = outs[bi].ap()
                    for co in range(2):
                        mm, mg = mgroup()
                        for c in (0, 1):
                            wino_mms(mm, vt[c], udb, 24 * co + 12 * c,
                                     start=c == 0,
                                     stop_banks=(0, 1, 2, 3) if c == 1 else ())
                        brow = 8 + bi * 2 + co
                        oe = opool.tile([128, 4, 128], dt.float32, tag="oe",
                                        name="oe")
                        oo = opool.tile([128, 4, 128], dt.float32, tag="oo",
                                        name="oo")
                        inv_group(mg, brow, oe[:], oo[:])
                        for parity, ot in ((0, oe), (1, oo)):
                            nc.sync.dma_start(
                                orr[co * 128:(co + 1) * 128,
                                    4 * j:4 * j + 4, parity, :], ot[:])

                for p, k in enumerate(order):
                    stageA(k, p)
                    if p >= 1:
                        stageC(order[p - 1])
                    if p >= 2:
                        stageD(order[p - 2])
                stageC(order[-1])
                stageD(order[-2])
                stageD(order[-1])

            branch(0, desc=True)
            branch(1, desc=False)

    nc.compile()
    return nc


_NC_CACHE = {}


def _get_nc(H):
    if H not in _NC_CACHE:
        _NC_CACHE[H] = _build(H)
    return _NC_CACHE[H]


def kernel(**inputs):
    from concourse import bass_utils

    x = np.asarray(inputs["x"], np.float32)
    B, C, H, W = x.shape
    assert (C, W) == (256, 128) and H % 8 == 0

    shared = _prep_host(inputs)
    nc = _get_nc(H)

    in_maps = []
    for b in range(B):
        m = dict(shared)
        m["xpad"] = _pad_x_sample(x[b], H)
        in_maps.append(m)

    import os
    trace = bool(int(os.environ.get("KERNEL_TRACE", "0")))
    res = bass_utils.run_bass_kernel_spmd(
        nc, in_maps, core_ids=list(range(B)), trace=trace)
    kernel.last_result = res

    otl = np.stack([res.results[b]["out_tl"].reshape(256, H, 128)
                    for b in range(B)])
    obr = np.stack([res.results[b]["out_br"].reshape(256, H, 128)
                    for b in range(B)])
    return otl, obr


# revision 6
# speedup vs baseline: 1.0002x; 1.0002x over previous
"""CornerPool kernel for Trainium2 — fused 1D Winograd F(2,3) along H, bf16.

One sample per NeuronCore (B=8). All 3x3 convs use Winograd F(2,3) on the
H axis (2 output rows per tile, taps along W stay direct): per output
chunk of 8 rows, 4 PSUM banks accumulate M_u = sum_{ci,dx} U_u^T V_u with
U_u = G-transformed (BN-folded) weights; DVE combines y_even=M0+M1+M2,
y_odd=M1-M2-M3; ScalarE applies bias+ReLU. The 1x1 convs of stage C are
folded into the M0 (+w1) and M3 (-w1) accumulations, so they ride the
same inverse. Corner pools: H pools via in-place shifted-max doubling
on DVE + inter-chunk carry; W pools via DVE prefix-scan per row. A short
burst of dummy matmuls on zeroed scratch warms the PE clock during the
preamble DMA wait. The whole net runs fused in
SBUF (two directional passes: TL descending, BR ascending) — x, weights
and rolling sum/tl windows stay on-chip; only x/weights in and outputs
out touch DRAM.
"""

import numpy as np

_P = 128
_CH = 16          # chunks per image; chunk = 8 image rows = 4 Winograd tiles
_G = np.array([[1, 0, 0], [0.5, 0.5, 0.5], [0.5, -0.5, 0.5], [0, 0, 1]],
              np.float32)


def _bf16():
    import ml_dtypes
    return ml_dtypes.bfloat16


def _prep_host(inputs):
    """Fold BN scales, G-transform weights along dy, build bf16 lhsT arrays."""
    f32 = np.float32
    BF = _bf16()

    def scaled(name):
        w = np.asarray(inputs["w_" + name], f32)
        s = np.asarray(inputs["s_" + name], f32)
        return w * s[:, None, None, None]

    def bias(name):
        return np.asarray(inputs["b_" + name], f32)

    def gtrans(w):
        # w [co, ci, 3, 3] -> [ci, 4u, 3dx, co]
        return np.einsum('uy,oiyx->iuxo', _G, w).astype(f32)

    # stage A: [ci=256, 4, 3, co=128] -> [4conv][128k, 2ci*12, 128m]
    def layA(w):
        a = gtrans(w).reshape(2, 128, 12, 128)
        return np.ascontiguousarray(a.transpose(1, 0, 2, 3).reshape(128, 24, 128))

    ua = np.stack([layA(scaled(n)) for n in ("t", "l", "b", "r")]).astype(BF)

    # stage C3: [ci=128, 4, 3, co=256] -> [2br][128k, 2co*12, 128m]
    def layC(w3):
        a = gtrans(w3).reshape(128, 12, 2, 128)
        return np.ascontiguousarray(a.transpose(0, 2, 1, 3).reshape(128, 24, 128))

    uc = np.stack([layC(scaled("tl3")), layC(scaled("br3"))]).astype(BF)

    # stage C1: [co=256, ci=256] -> [2br][128k, co_t*4 + sign*2 + ci_t, 128m]
    def layC1(w1):
        a = w1[:, :, 0, 0].T.reshape(2, 128, 2, 128)   # ci_t, k, co_t, m
        both = np.stack([a, -a], axis=0)               # sign, ci_t, k, co_t, m
        return np.ascontiguousarray(
            both.transpose(2, 3, 0, 1, 4).reshape(128, 8, 128))

    w1 = np.stack([layC1(scaled("tl1")), layC1(scaled("br1"))]).astype(BF)

    # stage D: [ci=256, 4, 3, co=256] -> [2br][128k, co_t*24 + ci_t*12 + uxdx, 128m]
    def layD(w):
        a = gtrans(w).reshape(2, 128, 12, 2, 128)      # ci_t, k, uxdx, co_t, m
        return np.ascontiguousarray(
            a.transpose(1, 3, 0, 2, 4).reshape(128, 48, 128))

    ud = np.stack([layD(scaled("tlo")), layD(scaled("bro"))]).astype(BF)

    bias_rows = [bias("t"), bias("l"), bias("b"), bias("r")]
    for n3, n1 in (("tl3", "tl1"), ("br3", "br1")):
        comb = bias(n3) + bias(n1)
        bias_rows += [comb[:128], comb[128:]]
    for n in ("tlo", "bro"):
        bb = bias(n)
        bias_rows += [bb[:128], bb[128:]]
    bias_all = np.ascontiguousarray(np.stack(bias_rows).T).astype(f32)

    return {"ua": ua, "uc": uc, "w1": w1, "ud": ud, "bias": bias_all}


def _pad_x_sample(xs, H):
    """[256, H, 128] f32 -> [2, 128, H+2, 130] bf16 zero-padded."""
    BF = _bf16()
    xp = np.zeros((2, 128, H + 2, 130), BF)
    xp[:, :, 1:H + 1, 1:129] = xs.reshape(2, 128, H, 128).astype(BF)
    return xp


def _build(H):
    import concourse.bacc as bacc
    import concourse.mybir as mybir
    import concourse.tile as tile
    import contextlib

    dt = mybir.dt
    Alu = mybir.AluOpType
    Act = mybir.ActivationFunctionType
    BF = dt.bfloat16
    CH = H // 8
    HP = H + 2

    nc = bacc.Bacc("TRN2", target_bir_lowering=False, debug=False)

    xpad = nc.dram_tensor("xpad", [2, 128, HP, 130], BF, kind="ExternalInput")
    ua_d = nc.dram_tensor("ua", [4, 128, 24, 128], BF, kind="ExternalInput")
    uc_d = nc.dram_tensor("uc", [2, 128, 24, 128], BF, kind="ExternalInput")
    w1_d = nc.dram_tensor("w1", [2, 128, 8, 128], BF, kind="ExternalInput")
    ud_d = nc.dram_tensor("ud", [2, 128, 48, 128], BF, kind="ExternalInput")
    bias_d = nc.dram_tensor("bias", [128, 12], dt.float32, kind="ExternalInput")
    # outputs declared row-parity-split: [co, jj, t, w] = [co, 2*jj + t, w]
    out_tl = nc.dram_tensor("out_tl", [256, H // 2, 2, 128], dt.float32,
                            kind="ExternalOutput")
    out_br = nc.dram_tensor("out_br", [256, H // 2, 2, 128], dt.float32,
                            kind="ExternalOutput")
    outs = [out_tl, out_br]

    with tile.TileContext(nc) as tc:
        with contextlib.ExitStack() as ctx:
            xpool = ctx.enter_context(tc.tile_pool(name="xp", bufs=1))
            upool = ctx.enter_context(tc.tile_pool(name="up", bufs=1))
            rpool = ctx.enter_context(tc.tile_pool(name="rp", bufs=1))
            vpool = ctx.enter_context(tc.tile_pool(name="vp", bufs=1))
            tpool = ctx.enter_context(tc.tile_pool(name="tp", bufs=2))
            ipool = ctx.enter_context(tc.tile_pool(name="ip", bufs=3))
            opool = ctx.enter_context(tc.tile_pool(name="op", bufs=2))
            cpool = ctx.enter_context(tc.tile_pool(name="cp", bufs=2))
            mpool = ctx.enter_context(tc.tile_pool(name="mp", bufs=1))
            pspool = ctx.enter_context(tc.tile_pool(name="ps", bufs=2,
                                                    space="PSUM"))

            # ---------------- preamble: x, weights, rings ----------------
            xt = [xpool.tile([128, HP, 130], BF, tag=f"x{c}", name=f"x{c}")
                  for c in (0, 1)]
            bt = mpool.tile([128, 12], dt.float32, tag="bias")
            nc.gpsimd.dma_start(bt[:], bias_d.ap())

            # PE warm-up: dummy matmuls on zeroed scratch fill the preamble
            # DMA wait so the HAM clock ramp is paid on idle time and the
            # first real matmuls run at full rate.
            nwarm = int(__import__("os").environ.get("NWARM", "21"))
            if nwarm:
                wsc = mpool.tile([128, 512], BF, tag="warm", name="wsc")
                nc.vector.memset(wsc[:].bitcast(dt.uint16), 0.0)
                wps = pspool.tile([128, 512], dt.float32, tag="m0",
                                  name="m0w")
                for _ in range(nwarm):
                    nc.tensor.matmul(wps[:], wsc[:, 0:128], wsc[:],
                                     start=True, stop=True)

            def load_wb(bi):
                """Load one branch's transformed weights into SBUF."""
                a_t, a_l = (0, 1) if bi == 0 else (2, 3)
                uat = upool.tile([128, 24, 128], BF, tag="uat", name="uat",
                                 bufs=2)
                ual = upool.tile([128, 24, 128], BF, tag="ual", name="ual",
                                 bufs=2)
                ucb = upool.tile([128, 24, 128], BF, tag="ucb", name="ucb")
                w1b = upool.tile([128, 8, 128], BF, tag="w1b", name="w1b")
                udb = upool.tile([128, 48, 128], BF, tag="udb", name="udb")
                nc.gpsimd.dma_start(uat[:], ua_d.ap()[a_t])
                nc.gpsimd.dma_start(ual[:], ua_d.ap()[a_l])
                nc.gpsimd.dma_start(ucb[:], uc_d.ap()[bi])
                nc.gpsimd.dma_start(w1b[:], w1_d.ap()[bi])
                nc.gpsimd.dma_start(udb[:], ud_d.ap()[bi])
                return {a_t: uat, a_l: ual}, ucb, w1b, udb

            # x lands in need-order for the first (descending) branch:
            # top 10 rows (first chunk) -> stage-A weights -> the rest in
            # progressively larger slices. The DMA pool drains serially, so
            # byte order here is the PE-start latency.
            for c in (0, 1):
                eng = nc.sync if c == 0 else nc.scalar
                eng.dma_start(xt[c][:, HP - 10:, :], xpad.ap()[c][:, HP - 10:, :])
            wb0 = load_wb(0)
            cuts = [HP - 10, HP - 42, HP - 74, 0]
            for i in range(len(cuts) - 1):
                a, b = cuts[i + 1], cuts[i]
                for c in (0, 1):
                    eng = nc.sync if c == 0 else nc.scalar
                    eng.dma_start(xt[c][:, a:b, :], xpad.ap()[c][:, a:b, :])

            # persistent ring tiles (10 rows = 8 + 2 halo), zeroed once
            st = [rpool.tile([128, 10, 130], BF, tag=f"s{r}", name=f"st{r}")
                  for r in range(3)]
            tlt = [[rpool.tile([128, 10, 130], BF, tag=f"t{co}{r}",
                               name=f"tlt{co}{r}")
                    for r in range(3)] for co in range(2)]
            for tile_ in st:
                nc.vector.memset(tile_[:].bitcast(dt.uint16), 0.0)
            for tile_ in tlt[0] + tlt[1]:
                nc.gpsimd.memset(tile_[:].bitcast(dt.uint16), 0.0)

            # parity views: rows (5,2): [p, j, par, w]
            def par(t10):
                return t10[:].rearrange("p (j t) w -> p j t w", t=2)

            xr = [t[:].rearrange("p (j t) w -> p j t w", t=2) for t in xt]

            def inv_group(mg, brow, dst_e, dst_o):
                """ACT evacuates the 4-bank M group in one copy to bf16 SBUF;
                DVE combines (2x); ACT applies bias+ReLU to both parities."""
                s = ipool.tile([128, 4, 512], BF, tag="ev", name="ev")
                for u in range(4):
                    nc.scalar.copy(s[:, u], mg[u][:])
                a = ipool.tile([128, 512], BF, tag="cmb0", name="cmb0")
                b = ipool.tile([128, 512], BF, tag="cmb1", name="cmb1")
                nc.vector.tensor_tensor(a[:], s[:, 0], s[:, 1], Alu.add)
                nc.vector.tensor_tensor(a[:], a[:], s[:, 2], Alu.add)
                nc.vector.tensor_tensor(b[:], s[:, 1], s[:, 2], Alu.subtract)
                nc.vector.tensor_tensor(b[:], b[:], s[:, 3], Alu.subtract)
                r = lambda t: t[:].rearrange("p (a b) -> p a b", b=128)
                nc.scalar.activation(dst_e, r(a), Act.Relu,
                                     bias=bt[:, brow:brow + 1], scale=1.0)
                nc.scalar.activation(dst_o, r(b), Act.Relu,
                                     bias=bt[:, brow:brow + 1], scale=1.0)

            def vmaps(src_par, j0, pfx, n=4):
                """4 V tiles [128, n, 130] from parity view at tile row j0."""
                vt = [vpool.tile([128, 4, 130], BF, tag=f"v{pfx}{u}",
                                 name=f"v{pfx}{u}")
                      for u in range(4)]
                e0 = src_par[:, j0:j0 + n, 0, :]
                o0 = src_par[:, j0:j0 + n, 1, :]
                e1 = src_par[:, j0 + 1:j0 + n + 1, 0, :]
                o1 = src_par[:, j0 + 1:j0 + n + 1, 1, :]
                nc.vector.tensor_tensor(vt[0][:, :n], e0, e1, Alu.subtract)
                nc.vector.tensor_tensor(vt[1][:, :n], o0, e1, Alu.add)
                nc.vector.tensor_tensor(vt[2][:, :n], e1, o0, Alu.subtract)
                nc.vector.tensor_tensor(vt[3][:, :n], o0, o1, Alu.subtract)
                return vt

            def mgroup():
                g = [pspool.tile([128, 512], dt.float32, tag=f"m{u}",
                                 name=f"m{u}")
                     for u in range(4)]
                return g, g

            def wino_mms(mm, vt, w, off, start, stop_banks):
                """Accumulate 4 banks: mm[u] += sum_dx w[:, off+u*3+dx]^T vt[u]."""
                for u in range(4):
                    for dx in range(3):
                        nc.tensor.matmul(
                            mm[u][:], w[:, off + u * 3 + dx],
                            vt[u][:, :, dx:dx + 128],
                            start=start and dx == 0,
                            stop=(u in stop_banks) and dx == 2)

            # ------------------------- branch pass -----------------------
            def branch(bi, desc):
                a_t, a_l = (0, 1) if bi == 0 else (2, 3)
                order = list(reversed(range(CH))) if desc else list(range(CH))
                ua2, ucb, w1b, udb = wb0 if bi == 0 else load_wb(1)
                carry = cpool.tile([128, 1, 128], BF, tag=f"cr{bi}")
                nc.vector.memset(carry[:].bitcast(dt.uint16), 0.0)

                def stageA(k, p):
                    nonlocal carry
                    # V of x for this chunk: padded rows 8k..8k+9 -> j0 = 4k
                    vx = [vmaps(xr[c], 4 * k, f"x{c}") for c in (0, 1)]
                    tl_tiles = []
                    for conv in (a_t, a_l):
                        mm, mg = mgroup()
                        for c in (0, 1):
                            wino_mms(mm, vx[c], ua2[conv], 12 * c,
                                     start=c == 0,
                                     stop_banks=(0, 1, 2, 3) if c == 1 else ())
                        tt = tpool.tile([128, 8, 130], BF, tag=f"ab{conv % 2}")
                        tpar = tt[:].rearrange("p (j t) w -> p j t w", t=2)
                        inv_group(mg, conv, tpar[:, 0:4, 0, 1:129],
                                  tpar[:, 0:4, 1, 1:129])
                        tl_tiles.append(tt)
                    tt, lt = tl_tiles
                    # H pool on tt: in-place shifted maxes (DVE streaming)
                    ti = tt[:, :, 1:129]
                    if desc:
                        nc.vector.tensor_tensor(ti[:, 0:7], ti[:, 0:7],
                                                ti[:, 1:8], Alu.max)
                        nc.vector.tensor_tensor(ti[:, 0:6], ti[:, 0:6],
                                                ti[:, 2:8], Alu.max)
                        nc.vector.tensor_tensor(ti[:, 0:4], ti[:, 0:4],
                                                ti[:, 4:8], Alu.max)
                    else:
                        nc.vector.tensor_tensor(ti[:, 1:8], ti[:, 1:8],
                                                ti[:, 0:7], Alu.max)
                        nc.vector.tensor_tensor(ti[:, 2:8], ti[:, 2:8],
                                                ti[:, 0:6], Alu.max)
                        nc.vector.tensor_tensor(ti[:, 4:8], ti[:, 4:8],
                                                ti[:, 0:4], Alu.max)
                    nc.vector.tensor_tensor(ti[:], ti[:],
                                            carry[:].broadcast_to([128, 8, 128]),
                                            Alu.max)
                    if p != CH - 1:
                        nxt = cpool.tile([128, 1, 128], BF, tag=f"cr{bi}")
                        csrc = ti[:, 0:1] if desc else ti[:, 7:8]
                        nc.vector.tensor_copy(nxt[:], csrc)
                        carry = nxt
                    # W pool on lt (reverse for TL, forward for BR)
                    for h in range(8):
                        v = lt[:, h, 1:129]
                        if bi == 0:
                            v = v[:, ::-1]
                        nc.vector.tensor_tensor_scan(v, v, v, 0.0,
                                                     op0=Alu.max, op1=Alu.bypass)
                    # sum -> ring tile k (rows 1..8 interior)
                    s = st[k % 3]
                    nc.vector.tensor_tensor(s[:, 1:9, 1:129], ti[:],
                                            lt[:, :, 1:129], Alu.add)
                    # halo exchange with previously-produced neighbor
                    nb = k + 1 if desc else k - 1
                    if p == 0:
                        edge = s[:, 9:10, :] if desc else s[:, 0:1, :]
                        nc.gpsimd.memset(edge.bitcast(dt.uint16), 0.0)
                    else:
                        nbt = st[nb % 3]
                        if desc:  # my row8 -> nb row0 ; nb row1 -> my row9
                            nc.vector.tensor_copy(nbt[:, 0:1, :], s[:, 8:9, :])
                            nc.vector.tensor_copy(s[:, 9:10, :], nbt[:, 1:2, :])
                        else:     # my row1 -> nb row9 ; nb row8 -> my row0
                            nc.vector.tensor_copy(nbt[:, 9:10, :], s[:, 1:2, :])
                            nc.vector.tensor_copy(s[:, 0:1, :], nbt[:, 8:9, :])
                    if p == CH - 1:
                        edge = s[:, 0:1, :] if desc else s[:, 9:10, :]
                        nc.gpsimd.memset(edge.bitcast(dt.uint16), 0.0)

                def stageC(j):
                    s = st[j % 3]
                    vs = vmaps(par(s), 0, "s")
                    for co in range(2):
                        mm, mg = mgroup()
                        wino_mms(mm, vs, ucb, 12 * co,
                                 start=True, stop_banks=(1, 2))
                        # fold C1: +w1 into m0 (even), -w1 into m3 (odd)
                        for c in (0, 1):
                            nc.tensor.matmul(
                                mm[0][:], w1b[:, co * 4 + 0 * 2 + c],
                                xr[c][:, 4 * j:4 * j + 4, 1, 1:129],
                                start=False, stop=c == 1)
                            nc.tensor.matmul(
                                mm[3][:], w1b[:, co * 4 + 1 * 2 + c],
                                xr[c][:, 4 * j + 1:4 * j + 5, 0, 1:129],
                                start=False, stop=c == 1)
                        d = tlt[co][j % 3]
                        dpar = par(d)
                        brow = 4 + bi * 2 + co
                        inv_group(mg, brow, dpar[:, 0:4, 1, 1:129],
                                  dpar[:, 1:5, 0, 1:129])
                    # halo exchange on tl ring
                    nb = j + 1 if desc else j - 1
                    first = (j == order[0])
                    last = (j == order[-1])
                    for co in range(2):
                        d = tlt[co][j % 3]
                        if first:
                            edge = d[:, 9:10, :] if desc else d[:, 0:1, :]
                            nc.gpsimd.memset(edge.bitcast(dt.uint16), 0.0)
                        else:
                            nbt = tlt[co][nb % 3]
                            if desc:
                                nc.vector.tensor_copy(nbt[:, 0:1, :], d[:, 8:9, :])
                                nc.vector.tensor_copy(d[:, 9:10, :], nbt[:, 1:2, :])
                            else:
                                nc.vector.tensor_copy(nbt[:, 9:10, :], d[:, 1:2, :])
                                nc.vector.tensor_copy(d[:, 0:1, :], nbt[:, 8:9, :])
                        if last:
                            edge = d[:, 0:1, :] if desc else d[:, 9:10, :]
                            nc.gpsimd.memset(edge.bitcast(dt.uint16), 0.0)

                def stageD(j):
                    vt = [vmaps(par(tlt[c][j % 3]), 0, f"d{c}") for c in (0, 1)]
                    orr = outs[bi].ap()
                    for co in range(2):
                        mm, mg = mgroup()
                        for c in (0, 1):
                            wino_mms(mm, vt[c], udb, 24 * co + 12 * c,
                                     start=c == 0,
                                     stop_banks=(0, 1, 2, 3) if c == 1 else ())
                        brow = 8 + bi * 2 + co
                        oe = opool.tile([128, 4, 128], dt.float32, tag="oe",
                                        name="oe")
                        oo = opool.tile([128, 4, 128], dt.float32, tag="oo",
                                        name="oo")
                        inv_group(mg, brow, oe[:], oo[:])
                        for parity, ot in ((0, oe), (1, oo)):
                            nc.sync.dma_start(
                                orr[co * 128:(co + 1) * 128,
                                    4 * j:4 * j + 4, parity, :], ot[:])

                for p, k in enumerate(order):
                    stageA(k, p)
                    if p >= 1:
                        stageC(order[p - 1])
                    if p >= 2:
                        stageD(order[p - 2])
                stageC(order[-1])
                stageD(order[-2])
                stageD(order[-1])

            branch(0, desc=True)
            branch(1, desc=False)

    nc.compile()
    return nc


_NC_CACHE = {}


def _get_nc(H):
    if H not in _NC_CACHE:
        _NC_CACHE[H] = _build(H)
    return _NC_CACHE[H]


def kernel(**inputs):
    from concourse import bass_utils

    x = np.asarray(inputs["x"], np.float32)
    B, C, H, W = x.shape
    assert (C, W) == (256, 128) and H % 8 == 0

    shared = _prep_host(inputs)
    nc = _get_nc(H)

    in_maps = []
    for b in range(B):
        m = dict(shared)
        m["xpad"] = _pad_x_sample(x[b], H)
        in_maps.append(m)

    import os
    trace = bool(int(os.environ.get("KERNEL_TRACE", "0")))
    res = bass_utils.run_bass_kernel_spmd(
        nc, in_maps, core_ids=list(range(B)), trace=trace)
    kernel.last_result = res

    otl = np.stack([res.results[b]["out_tl"].reshape(256, H, 128)
                    for b in range(B)])
    obr = np.stack([res.results[b]["out_br"].reshape(256, H, 128)
                    for b in range(B)])
    return otl, obr


# revision 7
# speedup vs baseline: 1.0003x; 1.0001x over previous
"""CornerPool kernel for Trainium2 — fused 1D Winograd F(2,3) along H, bf16.

One sample per NeuronCore (B=8). All 3x3 convs use Winograd F(2,3) on the
H axis (2 output rows per tile, taps along W stay direct): per output
chunk of 8 rows, 4 PSUM banks accumulate M_u = sum_{ci,dx} U_u^T V_u with
U_u = G-transformed (BN-folded) weights; DVE combines y_even=M0+M1+M2,
y_odd=M1-M2-M3; ScalarE applies bias+ReLU. The 1x1 convs of stage C are
folded into the M0 (+w1) and M3 (-w1) accumulations, so they ride the
same inverse. Corner pools: H pools via in-place shifted-max doubling
on DVE + inter-chunk carry; W pools via DVE prefix-scan per row. A short
burst of dummy matmuls on zeroed scratch warms the PE clock during the
preamble DMA wait. The whole net runs fused in
SBUF (two directional passes: TL descending, BR ascending) — x, weights
and rolling sum/tl windows stay on-chip; only x/weights in and outputs
out touch DRAM.
"""

import numpy as np

_P = 128
_CH = 16          # chunks per image; chunk = 8 image rows = 4 Winograd tiles
_G = np.array([[1, 0, 0], [0.5, 0.5, 0.5], [0.5, -0.5, 0.5], [0, 0, 1]],
              np.float32)


def _bf16():
    import ml_dtypes
    return ml_dtypes.bfloat16


def _prep_host(inputs):
    """Fold BN scales, G-transform weights along dy, build bf16 lhsT arrays."""
    f32 = np.float32
    BF = _bf16()

    def scaled(name):
        w = np.asarray(inputs["w_" + name], f32)
        s = np.asarray(inputs["s_" + name], f32)
        return w * s[:, None, None, None]

    def bias(name):
        return np.asarray(inputs["b_" + name], f32)

    def gtrans(w):
        # w [co, ci, 3, 3] -> [ci, 4u, 3dx, co]
        return np.einsum('uy,oiyx->iuxo', _G, w).astype(f32)

    # stage A: [ci=256, 4, 3, co=128] -> [4conv][128k, 2ci*12, 128m]
    def layA(w):
        a = gtrans(w).reshape(2, 128, 12, 128)
        return np.ascontiguousarray(a.transpose(1, 0, 2, 3).reshape(128, 24, 128))

    ua = np.stack([layA(scaled(n)) for n in ("t", "l", "b", "r")]).astype(BF)

    # stage C3: [ci=128, 4, 3, co=256] -> [2br][128k, 2co*12, 128m]
    def layC(w3):
        a = gtrans(w3).reshape(128, 12, 2, 128)
        return np.ascontiguousarray(a.transpose(0, 2, 1, 3).reshape(128, 24, 128))

    uc = np.stack([layC(scaled("tl3")), layC(scaled("br3"))]).astype(BF)

    # stage C1: [co=256, ci=256] -> [2br][128k, co_t*4 + sign*2 + ci_t, 128m]
    def layC1(w1):
        a = w1[:, :, 0, 0].T.reshape(2, 128, 2, 128)   # ci_t, k, co_t, m
        both = np.stack([a, -a], axis=0)               # sign, ci_t, k, co_t, m
        return np.ascontiguousarray(
            both.transpose(2, 3, 0, 1, 4).reshape(128, 8, 128))

    w1 = np.stack([layC1(scaled("tl1")), layC1(scaled("br1"))]).astype(BF)

    # stage D: [ci=256, 4, 3, co=256] -> [2br][128k, co_t*24 + ci_t*12 + uxdx, 128m]
    def layD(w):
        a = gtrans(w).reshape(2, 128, 12, 2, 128)      # ci_t, k, uxdx, co_t, m
        return np.ascontiguousarray(
            a.transpose(1, 3, 0, 2, 4).reshape(128, 48, 128))

    ud = np.stack([layD(scaled("tlo")), layD(scaled("bro"))]).astype(BF)

    bias_rows = [bias("t"), bias("l"), bias("b"), bias("r")]
    for n3, n1 in (("tl3", "tl1"), ("br3", "br1")):
        comb = bias(n3) + bias(n1)
        bias_rows += [comb[:128], comb[128:]]
    for n in ("tlo", "bro"):
        bb = bias(n)
        bias_rows += [bb[:128], bb[128:]]
    bias_all = np.ascontiguousarray(np.stack(bias_rows).T).astype(f32)

    return {"ua": ua, "uc": uc, "w1": w1, "ud": ud, "bias": bias_all}


def _pad_x_sample(xs, H):
    """[256, H, 128] f32 -> [2, 128, H+2, 130] bf16 zero-padded."""
    BF = _bf16()
    xp = np.zeros((2, 128, H + 2, 130), BF)
    xp[:, :, 1:H + 1, 1:129] = xs.reshape(2, 128, H, 128).astype(BF)
    return xp


def _build(H):
    import concourse.bacc as bacc
    import concourse.mybir as mybir
    import concourse.tile as tile
    import contextlib

    dt = mybir.dt
    Alu = mybir.AluOpType
    Act = mybir.ActivationFunctionType
    BF = dt.bfloat16
    CH = H // 8
    HP = H + 2

    nc = bacc.Bacc("TRN2", target_bir_lowering=False, debug=False)

    xpad = nc.dram_tensor("xpad", [2, 128, HP, 130], BF, kind="ExternalInput")
    ua_d = nc.dram_tensor("ua", [4, 128, 24, 128], BF, kind="ExternalInput")
    uc_d = nc.dram_tensor("uc", [2, 128, 24, 128], BF, kind="ExternalInput")
    w1_d = nc.dram_tensor("w1", [2, 128, 8, 128], BF, kind="ExternalInput")
    ud_d = nc.dram_tensor("ud", [2, 128, 48, 128], BF, kind="ExternalInput")
    bias_d = nc.dram_tensor("bias", [128, 12], dt.float32, kind="ExternalInput")
    # outputs declared row-parity-split: [co, jj, t, w] = [co, 2*jj + t, w]
    out_tl = nc.dram_tensor("out_tl", [256, H // 2, 2, 128], dt.float32,
                            kind="ExternalOutput")
    out_br = nc.dram_tensor("out_br", [256, H // 2, 2, 128], dt.float32,
                            kind="ExternalOutput")
    outs = [out_tl, out_br]

    with tile.TileContext(nc) as tc:
        with contextlib.ExitStack() as ctx:
            xpool = ctx.enter_context(tc.tile_pool(name="xp", bufs=1))
            upool = ctx.enter_context(tc.tile_pool(name="up", bufs=1))
            rpool = ctx.enter_context(tc.tile_pool(name="rp", bufs=1))
            vpool = ctx.enter_context(tc.tile_pool(name="vp", bufs=1))
            tpool = ctx.enter_context(tc.tile_pool(name="tp", bufs=2))
            ipool = ctx.enter_context(tc.tile_pool(name="ip", bufs=4))
            opool = ctx.enter_context(tc.tile_pool(name="op", bufs=2))
            cpool = ctx.enter_context(tc.tile_pool(name="cp", bufs=2))
            mpool = ctx.enter_context(tc.tile_pool(name="mp", bufs=1))
            pspool = ctx.enter_context(tc.tile_pool(name="ps", bufs=2,
                                                    space="PSUM"))

            # ---------------- preamble: x, weights, rings ----------------
            xt = [xpool.tile([128, HP, 130], BF, tag=f"x{c}", name=f"x{c}")
                  for c in (0, 1)]
            bt = mpool.tile([128, 12], dt.float32, tag="bias")
            nc.gpsimd.dma_start(bt[:], bias_d.ap())

            # PE warm-up: dummy matmuls on zeroed scratch fill the preamble
            # DMA wait so the HAM clock ramp is paid on idle time and the
            # first real matmuls run at full rate.
            nwarm = int(__import__("os").environ.get("NWARM", "21"))
            if nwarm:
                wsc = mpool.tile([128, 512], BF, tag="warm", name="wsc")
                nc.vector.memset(wsc[:].bitcast(dt.uint16), 0.0)
                wps = pspool.tile([128, 512], dt.float32, tag="m0",
                                  name="m0w")
                for _ in range(nwarm):
                    nc.tensor.matmul(wps[:], wsc[:, 0:128], wsc[:],
                                     start=True, stop=True)

            def load_wb(bi):
                """Load one branch's transformed weights into SBUF."""
                a_t, a_l = (0, 1) if bi == 0 else (2, 3)
                uat = upool.tile([128, 24, 128], BF, tag="uat", name="uat",
                                 bufs=2)
                ual = upool.tile([128, 24, 128], BF, tag="ual", name="ual",
                                 bufs=2)
                ucb = upool.tile([128, 24, 128], BF, tag="ucb", name="ucb")
                w1b = upool.tile([128, 8, 128], BF, tag="w1b", name="w1b")
                udb = upool.tile([128, 48, 128], BF, tag="udb", name="udb")
                nc.gpsimd.dma_start(uat[:], ua_d.ap()[a_t])
                nc.gpsimd.dma_start(ual[:], ua_d.ap()[a_l])
                nc.gpsimd.dma_start(ucb[:], uc_d.ap()[bi])
                nc.gpsimd.dma_start(w1b[:], w1_d.ap()[bi])
                nc.gpsimd.dma_start(udb[:], ud_d.ap()[bi])
                return {a_t: uat, a_l: ual}, ucb, w1b, udb

            # x lands in need-order for the first (descending) branch:
            # top 10 rows (first chunk) -> stage-A weights -> the rest in
            # progressively larger slices. The DMA pool drains serially, so
            # byte order here is the PE-start latency.
            for c in (0, 1):
                eng = nc.sync if c == 0 else nc.scalar
                eng.dma_start(xt[c][:, HP - 10:, :], xpad.ap()[c][:, HP - 10:, :])
            wb0 = load_wb(0)
            cuts = [HP - 10, HP - 42, HP - 74, 0]
            for i in range(len(cuts) - 1):
                a, b = cuts[i + 1], cuts[i]
                for c in (0, 1):
                    eng = nc.sync if c == 0 else nc.scalar
                    eng.dma_start(xt[c][:, a:b, :], xpad.ap()[c][:, a:b, :])

            # persistent ring tiles (10 rows = 8 + 2 halo), zeroed once
            st = [rpool.tile([128, 10, 130], BF, tag=f"s{r}", name=f"st{r}")
                  for r in range(3)]
            tlt = [[rpool.tile([128, 10, 130], BF, tag=f"t{co}{r}",
                               name=f"tlt{co}{r}")
                    for r in range(3)] for co in range(2)]
            for tile_ in st:
                nc.vector.memset(tile_[:].bitcast(dt.uint16), 0.0)
            for tile_ in tlt[0] + tlt[1]:
                nc.gpsimd.memset(tile_[:].bitcast(dt.uint16), 0.0)

            # parity views: rows (5,2): [p, j, par, w]
            def par(t10):
                return t10[:].rearrange("p (j t) w -> p j t w", t=2)

            xr = [t[:].rearrange("p (j t) w -> p j t w", t=2) for t in xt]

            def inv_group(mg, brow, dst_e, dst_o):
                """ACT evacuates the 4-bank M group in one copy to bf16 SBUF;
                DVE combines (2x); ACT applies bias+ReLU to both parities."""
                s = ipool.tile([128, 4, 512], BF, tag="ev", name="ev")
                for u in range(4):
                    nc.scalar.copy(s[:, u], mg[u][:])
                a = ipool.tile([128, 512], BF, tag="cmb0", name="cmb0")
                b = ipool.tile([128, 512], BF, tag="cmb1", name="cmb1")
                nc.vector.tensor_tensor(a[:], s[:, 0], s[:, 1], Alu.add)
                nc.vector.tensor_tensor(a[:], a[:], s[:, 2], Alu.add)
                nc.vector.tensor_tensor(b[:], s[:, 1], s[:, 2], Alu.subtract)
                nc.vector.tensor_tensor(b[:], b[:], s[:, 3], Alu.subtract)
                r = lambda t: t[:].rearrange("p (a b) -> p a b", b=128)
                nc.scalar.activation(dst_e, r(a), Act.Relu,
                                     bias=bt[:, brow:brow + 1], scale=1.0)
                nc.scalar.activation(dst_o, r(b), Act.Relu,
                                     bias=bt[:, brow:brow + 1], scale=1.0)

            def vmaps(src_par, j0, pfx, n=4):
                """4 V tiles [128, n, 130] from parity view at tile row j0."""
                vt = [vpool.tile([128, 4, 130], BF, tag=f"v{pfx}{u}",
                                 name=f"v{pfx}{u}")
                      for u in range(4)]
                e0 = src_par[:, j0:j0 + n, 0, :]
                o0 = src_par[:, j0:j0 + n, 1, :]
                e1 = src_par[:, j0 + 1:j0 + n + 1, 0, :]
                o1 = src_par[:, j0 + 1:j0 + n + 1, 1, :]
                nc.vector.tensor_tensor(vt[0][:, :n], e0, e1, Alu.subtract)
                nc.vector.tensor_tensor(vt[1][:, :n], o0, e1, Alu.add)
                nc.vector.tensor_tensor(vt[2][:, :n], e1, o0, Alu.subtract)
                nc.vector.tensor_tensor(vt[3][:, :n], o0, o1, Alu.subtract)
                return vt

            def mgroup():
                g = [pspool.tile([128, 512], dt.float32, tag=f"m{u}",
                                 name=f"m{u}")
                     for u in range(4)]
                return g, g

            def wino_mms(mm, vt, w, off, start, stop_banks):
                """Accumulate 4 banks: mm[u] += sum_dx w[:, off+u*3+dx]^T vt[u]."""
                for u in range(4):
                    for dx in range(3):
                        nc.tensor.matmul(
                            mm[u][:], w[:, off + u * 3 + dx],
                            vt[u][:, :, dx:dx + 128],
                            start=start and dx == 0,
                            stop=(u in stop_banks) and dx == 2)

            # ------------------------- branch pass -----------------------
            def branch(bi, desc):
                a_t, a_l = (0, 1) if bi == 0 else (2, 3)
                order = list(reversed(range(CH))) if desc else list(range(CH))
                ua2, ucb, w1b, udb = wb0 if bi == 0 else load_wb(1)
                carry = cpool.tile([128, 1, 128], BF, tag=f"cr{bi}")
                nc.vector.memset(carry[:].bitcast(dt.uint16), 0.0)

                def stageA(k, p):
                    nonlocal carry
                    # V of x for this chunk: padded rows 8k..8k+9 -> j0 = 4k
                    vx = [vmaps(xr[c], 4 * k, f"x{c}") for c in (0, 1)]
                    tl_tiles = []
                    for conv in (a_t, a_l):
                        mm, mg = mgroup()
                        for c in (0, 1):
                            wino_mms(mm, vx[c], ua2[conv], 12 * c,
                                     start=c == 0,
                                     stop_banks=(0, 1, 2, 3) if c == 1 else ())
                        tt = tpool.tile([128, 8, 130], BF, tag=f"ab{conv % 2}")
                        tpar = tt[:].rearrange("p (j t) w -> p j t w", t=2)
                        inv_group(mg, conv, tpar[:, 0:4, 0, 1:129],
                                  tpar[:, 0:4, 1, 1:129])
                        tl_tiles.append(tt)
                    tt, lt = tl_tiles
                    # H pool on tt: in-place shifted maxes (DVE streaming)
                    ti = tt[:, :, 1:129]
                    if desc:
                        nc.vector.tensor_tensor(ti[:, 0:7], ti[:, 0:7],
                                                ti[:, 1:8], Alu.max)
                        nc.vector.tensor_tensor(ti[:, 0:6], ti[:, 0:6],
                                                ti[:, 2:8], Alu.max)
                        nc.vector.tensor_tensor(ti[:, 0:4], ti[:, 0:4],
                                                ti[:, 4:8], Alu.max)
                    else:
                        nc.vector.tensor_tensor(ti[:, 1:8], ti[:, 1:8],
                                                ti[:, 0:7], Alu.max)
                        nc.vector.tensor_tensor(ti[:, 2:8], ti[:, 2:8],
                                                ti[:, 0:6], Alu.max)
                        nc.vector.tensor_tensor(ti[:, 4:8], ti[:, 4:8],
                                                ti[:, 0:4], Alu.max)
                    nc.vector.tensor_tensor(ti[:], ti[:],
                                            carry[:].broadcast_to([128, 8, 128]),
                                            Alu.max)
                    if p != CH - 1:
                        nxt = cpool.tile([128, 1, 128], BF, tag=f"cr{bi}")
                        csrc = ti[:, 0:1] if desc else ti[:, 7:8]
                        nc.vector.tensor_copy(nxt[:], csrc)
                        carry = nxt
                    # W pool on lt (reverse for TL, forward for BR)
                    for h in range(8):
                        v = lt[:, h, 1:129]
                        if bi == 0:
                            v = v[:, ::-1]
                        nc.vector.tensor_tensor_scan(v, v, v, 0.0,
                                                     op0=Alu.max, op1=Alu.bypass)
                    # sum -> ring tile k (rows 1..8 interior)
                    s = st[k % 3]
                    nc.vector.tensor_tensor(s[:, 1:9, 1:129], ti[:],
                                            lt[:, :, 1:129], Alu.add)
                    # halo exchange with previously-produced neighbor
                    nb = k + 1 if desc else k - 1
                    if p == 0:
                        edge = s[:, 9:10, :] if desc else s[:, 0:1, :]
                        nc.gpsimd.memset(edge.bitcast(dt.uint16), 0.0)
                    else:
                        nbt = st[nb % 3]
                        if desc:  # my row8 -> nb row0 ; nb row1 -> my row9
                            nc.vector.tensor_copy(nbt[:, 0:1, :], s[:, 8:9, :])
                            nc.vector.tensor_copy(s[:, 9:10, :], nbt[:, 1:2, :])
                        else:     # my row1 -> nb row9 ; nb row8 -> my row0
                            nc.vector.tensor_copy(nbt[:, 9:10, :], s[:, 1:2, :])
                            nc.vector.tensor_copy(s[:, 0:1, :], nbt[:, 8:9, :])
                    if p == CH - 1:
                        edge = s[:, 0:1, :] if desc else s[:, 9:10, :]
                        nc.gpsimd.memset(edge.bitcast(dt.uint16), 0.0)

                def stageC(j):
                    s = st[j % 3]
                    vs = vmaps(par(s), 0, "s")
                    for co in range(2):
                        mm, mg = mgroup()
                        wino_mms(mm, vs, ucb, 12 * co,
                                 start=True, stop_banks=(1, 2))
                        # fold C1: +w1 into m0 (even), -w1 into m3 (odd)
                        for c in (0, 1):
                            nc.tensor.matmul(
                                mm[0][:], w1b[:, co * 4 + 0 * 2 + c],
                                xr[c][:, 4 * j:4 * j + 4, 1, 1:129],
                                start=False, stop=c == 1)
                            nc.tensor.matmul(
                                mm[3][:], w1b[:, co * 4 + 1 * 2 + c],
                                xr[c][:, 4 * j + 1:4 * j + 5, 0, 1:129],
                                start=False, stop=c == 1)
                        d = tlt[co][j % 3]
                        dpar = par(d)
                        brow = 4 + bi * 2 + co
                        inv_group(mg, brow, dpar[:, 0:4, 1, 1:129],
                                  dpar[:, 1:5, 0, 1:129])
                    # halo exchange on tl ring
                    nb = j + 1 if desc else j - 1
                    first = (j == order[0])
                    last = (j == order[-1])
                    for co in range(2):
                        d = tlt[co][j % 3]
                        if first:
                            edge = d[:, 9:10, :] if desc else d[:, 0:1, :]
                            nc.gpsimd.memset(edge.bitcast(dt.uint16), 0.0)
                        else:
                            nbt = tlt[co][nb % 3]
                            if desc:
                                nc.vector.tensor_copy(nbt[:, 0:1, :], d[:, 8:9, :])
                                nc.vector.tensor_copy(d[:, 9:10, :], nbt[:, 1:2, :])
                            else:
                                nc.vector.tensor_copy(nbt[:, 9:10, :], d[:, 1:2, :])
                                nc.vector.tensor_copy(d[:, 0:1, :], nbt[:, 8:9, :])
                        if last:
                            edge = d[:, 0:1, :] if desc else d[:, 9:10, :]
                            nc.gpsimd.memset(edge.bitcast(dt.uint16), 0.0)

                def stageD(j):
                    vt = [vmaps(par(tlt[c][j % 3]), 0, f"d{c}") for c in (0, 1)]
                    orr = outs[bi].ap()
                    for co in range(2):
                        mm, mg = mgroup()
                        for c in (0, 1):
                            wino_mms(mm, vt[c], udb, 24 * co + 12 * c,
                                     start=c == 0,
                                     stop_banks=(0, 1, 2, 3) if c == 1 else ())
                        brow = 8 + bi * 2 + co
                        oe = opool.tile([128, 4, 128], dt.float32, tag="oe",
                                        name="oe")
                        oo = opool.tile([128, 4, 128], dt.float32, tag="oo",
                                        name="oo")
                        inv_group(mg, brow, oe[:], oo[:])
                        for parity, ot in ((0, oe), (1, oo)):
                            nc.sync.dma_start(
                                orr[co * 128:(co + 1) * 128,
                                    4 * j:4 * j + 4, parity, :], ot[:])

                for p, k in enumerate(order):
                    stageA(k, p)
                    if p >= 1:
                        stageC(order[p - 1])
                    if p >= 2:
                        stageD(order[p - 2])
                stageC(order[-1])
                stageD(order[-2])
                stageD(order[-1])

            branch(0, desc=True)
            branch(1, desc=False)

    nc.compile()
    return nc


_NC_CACHE = {}


def _get_nc(H):
    if H not in _NC_CACHE:
        _NC_CACHE[H] = _build(H)
    return _NC_CACHE[H]


def kernel(**inputs):
    from concourse import bass_utils

    x = np.asarray(inputs["x"], np.float32)
    B, C, H, W = x.shape
    assert (C, W) == (256, 128) and H % 8 == 0

    shared = _prep_host(inputs)
    nc = _get_nc(H)

    in_maps = []
    for b in range(B):
        m = dict(shared)
        m["xpad"] = _pad_x_sample(x[b], H)
        in_maps.append(m)

    import os
    trace = bool(int(os.environ.get("KERNEL_TRACE", "0")))
    res = bass_utils.run_bass_kernel_spmd(
        nc, in_maps, core_ids=list(range(B)), trace=trace)
    kernel.last_result = res

    otl = np.stack([res.results[b]["out_tl"].reshape(256, H, 128)
                    for b in range(B)])
    obr = np.stack([res.results[b]["out_br"].reshape(256, H, 128)
                    for b in range(B)])
    return otl, obr
